# revision 1
# baseline (speedup 1.0000x reference)
"""Trainium2 Bass kernel for CharacterLevelSpectral.

Math: the reference embeds chars (x = char/255; emb = x*W + b broadcast over D),
FFTs along seq, zeroes mid frequencies (keeps lowest k=S/4 and highest k), IFFTs,
takes the real part.  The whole pipeline is linear along seq and the bias is
constant along seq (a constant's spectrum lives at f=0, which the low-pass
keeps), so

    out[b, s, d] = y[b, s] * W[d] + b[d],   y = lowpass(char/255)

and the FFT only needs to run on the (B, S) scalar signal, not (B, S, D).

y is computed per batch row with a factorized N1=128 x N2=64 Cooley-Tukey
FFT -> mask -> IFFT, all as small fp32 matmuls on the TensorEngine plus two
elementwise twiddle stages on the VectorEngine.  The frequency mask only
depends on f2 (k = 2048 = 16*128), so the DFT_64/mask/IDFT_64 stage collapses
into one precomputed 64x64 complex matrix G.

The memory-bound part is materializing the (2, 8192, 256) fp32 output per
core (16.8 MB).  Each 128-row output chunk is produced by a single DVE
scalar_tensor_tensor op: (WPAT * y_col) + BPAT with y_col a per-partition
scalar, into 2MB staging tiles laid out so every partition's DMA descriptor
is 16KB contiguous in DRAM.

Sharding: batch dim across 8 cores (2 rows per core), no cross-core traffic.
"""

import numpy as np

import concourse.bass as bass
import concourse.mybir as mybir
import concourse.tile as tile
from concourse import bacc
from concourse.bass_utils import run_bass_kernel_spmd

B, S, D = 16, 8192, 256
NCORES = 8
BPC = B // NCORES  # batches per core
N1, N2 = 128, 64   # S = N1 * N2
KLP = S // 4       # low-pass cutoff

F32 = mybir.dt.float32
I32 = mybir.dt.int32
MULT = mybir.AluOpType.mult
ADD = mybir.AluOpType.add
SUB = mybir.AluOpType.subtract


def make_consts():
    """Input-independent DFT/twiddle constants (fp32)."""
    n1 = np.arange(N1)
    n2 = np.arange(N2)
    C128 = np.cos(2 * np.pi * np.outer(n1, n1) / N1)
    S128 = np.sin(2 * np.pi * np.outer(n1, n1) / N1)
    kept = np.r_[0 : KLP // N1, N2 - KLP // N1 : N2]
    diff = n2[None, :] - n2[:, None]  # [n2, m2']: m2' - n2
    G = sum(np.exp(2j * np.pi * diff * f2 / N2) for f2 in kept)
    c = {
        # MM1 rhs: A'[n2,f1] = Xm.T @ M1  (1/255 char normalization folded in)
        "m1re": C128 / 255.0,
        "m1im": -S128 / 255.0,
        # twiddle 1 (transposed layout [n2, f1]): exp(-2j pi f1 n2 / S)
        "twtre": np.cos(2 * np.pi * np.outer(n2, n1) / S),
        "twtim": -np.sin(2 * np.pi * np.outer(n2, n1) / S),
        # combined DFT64 -> mask -> IDFT64 [n2, m2']
        "gre": G.real,
        "gim": G.imag,
        "gimn": -G.imag,
        # twiddle 2 [f1, m2']: exp(+2j pi f1 m2' / S)
        "tw2re": np.cos(2 * np.pi * np.outer(n1, n2) / S),
        "tw2im": np.sin(2 * np.pi * np.outer(n1, n2) / S),
        # MM3 lhsT: Y' = M3_RE.T @ D_re + M3_IM_NEG.T @ D_im  (1/S ifft scale)
        "m3re": C128 / S,
        "m3imn": -S128 / S,
    }
    return {k: np.ascontiguousarray(v, dtype=np.float32) for k, v in c.items()}


def build_program():
    """Build the per-core SPMD Bass program (identical on all cores)."""
    nc = bacc.Bacc("TRN2", target_bir_lowering=False, debug=False)

    char_ext = nc.dram_tensor("char", [BPC, N1, N2], I32, kind="ExternalInput").ap()
    wrow_ext = nc.dram_tensor("wrow", [1, D], F32, kind="ExternalInput").ap()
    brow_ext = nc.dram_tensor("brow", [1, D], F32, kind="ExternalInput").ap()
    const_ext = {
        name: nc.dram_tensor(name, list(arr.shape), F32, kind="ExternalInput").ap()
        for name, arr in make_consts().items()
    }
    # out[b, p, t, f] with s = 64*p + 16*t + f//256, d = f%256  — row-major
    # identical to (BPC, S, D)
    out_ext = nc.dram_tensor("out", [BPC, N1, 4, 4096], F32, kind="ExternalOutput").ap()

    with tile.TileContext(nc) as tc:
        with (
            tc.tile_pool(name="consts", bufs=1) as cpool,
            tc.tile_pool(name="work", bufs=2) as wpool,
            tc.tile_pool(name="stg", bufs=3) as spool,
            tc.tile_pool(name="ppbc", bufs=1, space="PSUM") as ppbc,
            tc.tile_pool(name="ppa", bufs=1, space="PSUM") as ppa,
            tc.tile_pool(name="ppc", bufs=1, space="PSUM") as ppc,
            tc.tile_pool(name="ppy", bufs=2, space="PSUM") as ppy,
        ):
            # ---- constants into SBUF ----
            cs = {}
            for name, ext in const_ext.items():
                t = cpool.tile(list(ext.shape), F32, name=f"c_{name}", tag=f"c_{name}")
                nc.sync.dma_start(out=t[:], in_=ext)
                cs[name] = t
            wrow = cpool.tile([1, D], F32)
            nc.sync.dma_start(out=wrow[:], in_=wrow_ext)
            brow = cpool.tile([1, D], F32)
            nc.sync.dma_start(out=brow[:], in_=brow_ext)

            ones = cpool.tile([1, N1], F32)
            nc.gpsimd.memset(ones[:], 1.0)

            # WPAT/BPAT: W and bias rows broadcast to all 128 partitions via a
            # K=1 ones-matmul (PE) + copy out of PSUM.
            wpat_ps = ppbc.tile([N1, D], F32)
            nc.tensor.matmul(wpat_ps[:], ones[:], wrow[:], start=True, stop=True)
            wpat = cpool.tile([N1, D], F32)
            nc.vector.tensor_copy(wpat[:], wpat_ps[:])
            bpat_ps = ppbc.tile([N1, D], F32)
            nc.tensor.matmul(bpat_ps[:], ones[:], brow[:], start=True, stop=True)
            bpat = cpool.tile([N1, D], F32)
            nc.vector.tensor_copy(bpat[:], bpat_ps[:])

            for bb in range(BPC):
                # ---- load + int->float ----
                char_sb = wpool.tile([N1, N2], I32, tag="char_sb")
                nc.sync.dma_start(out=char_sb[:], in_=char_ext[bb])
                xf = wpool.tile([N1, N2], F32, tag="xf")
                nc.vector.tensor_copy(xf[:], char_sb[:])

                # ---- MM1: A'[n2, f1] = Xm.T @ M1 ----
                are = ppa.tile([N2, N1], F32, tag="are")
                nc.tensor.matmul(are[:], xf[:], cs["m1re"][:], start=True, stop=True)
                aim = ppa.tile([N2, N1], F32, tag="aim")
                nc.tensor.matmul(aim[:], xf[:], cs["m1im"][:], start=True, stop=True)

                # ---- twiddle 1: B' = A' * TWT (complex) ----
                t1 = wpool.tile([N2, N1], F32, tag="t1")
                nc.vector.tensor_tensor(t1[:], are[:], cs["twtre"][:], MULT)
                t2 = wpool.tile([N2, N1], F32, tag="t2")
                nc.vector.tensor_tensor(t2[:], aim[:], cs["twtim"][:], MULT)
                bre = wpool.tile([N2, N1], F32, tag="bre")
                nc.vector.tensor_tensor(bre[:], t1[:], t2[:], SUB)
                t3 = wpool.tile([N2, N1], F32, tag="t3")
                nc.vector.tensor_tensor(t3[:], are[:], cs["twtim"][:], MULT)
                t4 = wpool.tile([N2, N1], F32, tag="t4")
                nc.vector.tensor_tensor(t4[:], aim[:], cs["twtre"][:], MULT)
                bim = wpool.tile([N2, N1], F32, tag="bim")
                nc.vector.tensor_tensor(bim[:], t3[:], t4[:], ADD)

                # ---- MM2: Ck[f1, m2'] = B'.T @ G (complex) ----
                ckre = ppc.tile([N1, N2], F32, tag="ckre")
                nc.tensor.matmul(ckre[:], bre[:], cs["gre"][:], start=True, stop=False)
                nc.tensor.matmul(ckre[:], bim[:], cs["gimn"][:], start=False, stop=True)
                ckim = ppc.tile([N1, N2], F32, tag="ckim")
                nc.tensor.matmul(ckim[:], bre[:], cs["gim"][:], start=True, stop=False)
                nc.tensor.matmul(ckim[:], bim[:], cs["gre"][:], start=False, stop=True)

                # ---- twiddle 2: Dm = Ck * TW2 (complex) ----
                u1 = wpool.tile([N1, N2], F32, tag="u1")
                nc.vector.tensor_tensor(u1[:], ckre[:], cs["tw2re"][:], MULT)
                u2 = wpool.tile([N1, N2], F32, tag="u2")
                nc.vector.tensor_tensor(u2[:], ckim[:], cs["tw2im"][:], MULT)
                dre = wpool.tile([N1, N2], F32, tag="dre")
                nc.vector.tensor_tensor(dre[:], u1[:], u2[:], SUB)
                u3 = wpool.tile([N1, N2], F32, tag="u3")
                nc.vector.tensor_tensor(u3[:], ckre[:], cs["tw2im"][:], MULT)
                u4 = wpool.tile([N1, N2], F32, tag="u4")
                nc.vector.tensor_tensor(u4[:], ckim[:], cs["tw2re"][:], MULT)
                dim = wpool.tile([N1, N2], F32, tag="dim")
                nc.vector.tensor_tensor(dim[:], u3[:], u4[:], ADD)

                # ---- MM3: Y'[m1', m2'] (real part), y[64*m1' + m2'] ----
                ypsum = ppy.tile([N1, N2], F32, tag="ypsum")
                nc.tensor.matmul(ypsum[:], cs["m3re"][:], dre[:], start=True, stop=False)
                nc.tensor.matmul(ypsum[:], cs["m3imn"][:], dim[:], start=False, stop=True)
                ysc = wpool.tile([N1, N2], F32, tag="ysc")
                nc.vector.tensor_copy(ysc[:], ypsum[:])

                # ---- broadcast: out[p, j*256+d] = y[64p+j]*W[d] + b[d] ----
                for t in range(4):
                    stg = spool.tile([N1, 16 * D], F32, tag="stg")
                    for jl in range(16):
                        j = 16 * t + jl
                        nc.vector.scalar_tensor_tensor(
                            stg[:, jl * D : (jl + 1) * D],
                            wpat[:],
                            ysc[:, j : j + 1],
                            bpat[:],
                            MULT,
                            ADD,
                        )
                    nc.sync.dma_start(out=out_ext[bb, :, t, :], in_=stg[:])

    nc.compile()
    return nc


_NC = None


def _get_nc():
    global _NC
    if _NC is None:
        _NC = build_program()
    return _NC


def make_in_maps(char_ids, W, b):
    char = np.ascontiguousarray(np.asarray(char_ids), dtype=np.int32)
    char = char.reshape(NCORES, BPC, N1, N2)
    wrow = np.ascontiguousarray(np.asarray(W, dtype=np.float32)[:, 0][None, :])
    brow = np.ascontiguousarray(np.asarray(b, dtype=np.float32)[None, :])
    consts = make_consts()
    in_maps = []
    for i in range(NCORES):
        m = {"char": char[i], "wrow": wrow, "brow": brow}
        m.update(consts)
        in_maps.append(m)
    return in_maps


def kernel(char_ids, W, b):
    nc = _get_nc()
    in_maps = make_in_maps(char_ids, W, b)
    res = run_bass_kernel_spmd(nc, in_maps, core_ids=list(range(NCORES)))
    parts = [r["out"].reshape(BPC, S, D) for r in res.results]
    return np.concatenate(parts, axis=0).astype(np.float32)


# revision 3
# speedup vs baseline: 1.1092x; 1.1092x over previous
"""Trainium2 Bass kernel for CharacterLevelSpectral.

Math: the reference embeds chars (x = char/255; emb = x*W + b broadcast over D),
FFTs along seq, zeroes mid frequencies (keeps lowest k=S/4 and highest k), IFFTs,
takes the real part.  The whole pipeline is linear along seq and the bias is
constant along seq (a constant's spectrum lives at f=0, which the low-pass
keeps), so

    out[b, s, d] = y[b, s] * W[d] + b[d],   y = lowpass(char/255)

and the FFT only needs to run on the (B, S) scalar signal, not (B, S, D).

y is computed per batch row with a factorized N1=128 x N2=64 Cooley-Tukey
FFT -> mask -> IFFT, all as small fp32 matmuls on the TensorEngine plus two
elementwise twiddle stages on the VectorEngine.  The frequency mask only
depends on f2 (k = 2048 = 16*128), so the DFT_64/mask/IDFT_64 stage collapses
into one precomputed 64x64 complex matrix G.

The memory-bound part is materializing the (2, 8192, 256) fp32 output per
core (16.8 MB).  That broadcast (out_chunk = y_col x W + b) runs on the
TensorEngine as bf16 K=9 row-tiled matmuls: lhsT rows = 8 y-chunks + a ones
row, rhs = block-diagonal W replicas with a bias row (bias folded into the
matmul).  Matmul operands must sit on 32-aligned partition strips, so y
groups live at partition offsets {0,32,64,96} and the W/b constant is
replicated at the same offsets; row-tiled matmuls on different strips run
concurrently in the PE array.  The inverse-FFT stage emits y directly in
that strip layout: its input tile has 32-column strips whose 9th column is
(S, 0, 0, ...) so the IDFT matmul produces an exact ones row - no partition
shuffles anywhere.  PSUM->SBUF copies of the broadcast tiles alternate
between VectorE and ScalarE; 2MB staging tiles DMA out with 16KB-contiguous
per-partition descriptors.

Sharding: batch dim across 8 cores (2 rows per core), no cross-core traffic.
"""

import ml_dtypes
import numpy as np

import concourse.bass as bass
import concourse.mybir as mybir
import concourse.tile as tile
from concourse import bacc
from concourse.bass_utils import run_bass_kernel_spmd

B, S, D = 16, 8192, 256
NCORES = 8
BPC = B // NCORES  # batches per core
N1, N2 = 128, 64   # S = N1 * N2
KLP = S // 4       # low-pass cutoff
NG = 8             # chunks per broadcast group (K = NG + 1)

F32 = mybir.dt.float32
BF16 = mybir.dt.bfloat16
I32 = mybir.dt.int32
MULT = mybir.AluOpType.mult
ADD = mybir.AluOpType.add
SUB = mybir.AluOpType.subtract


def make_consts():
    """Input-independent DFT/twiddle constants (fp32)."""
    n1 = np.arange(N1)
    n2 = np.arange(N2)
    C128 = np.cos(2 * np.pi * np.outer(n1, n1) / N1)
    S128 = np.sin(2 * np.pi * np.outer(n1, n1) / N1)
    kept = np.r_[0 : KLP // N1, N2 - KLP // N1 : N2]
    diff = n2[None, :] - n2[:, None]  # [n2, m2']: m2' - n2
    G = sum(np.exp(2j * np.pi * diff * f2 / N2) for f2 in kept)
    c = {
        # MM1 rhs: A'[n2,f1] = Xm.T @ M1  (1/255 char normalization folded in)
        "m1re": C128 / 255.0,
        "m1im": -S128 / 255.0,
        # twiddle 1 (transposed layout [n2, f1]): exp(-2j pi f1 n2 / S)
        "twtre": np.cos(2 * np.pi * np.outer(n2, n1) / S),
        "twtim": -np.sin(2 * np.pi * np.outer(n2, n1) / S),
        # combined DFT64 -> mask -> IDFT64 [n2, m2']
        "gre": G.real,
        "gim": G.imag,
        "gimn": -G.imag,
        # twiddle 2 [f1, m2']: exp(+2j pi f1 m2' / S)
        "tw2re": np.cos(2 * np.pi * np.outer(n1, n2) / S),
        "tw2im": np.sin(2 * np.pi * np.outer(n1, n2) / S),
        # MM3 rhs: Yt = dm.T @ M3  (1/S ifft scale folded in)
        "m3re": C128 / S,
        "m3imn": -S128 / S,
    }
    return {k: np.ascontiguousarray(v, dtype=np.float32) for k, v in c.items()}


def build_program():
    """Build the per-core SPMD Bass program (identical on all cores)."""
    nc = bacc.Bacc("TRN2", target_bir_lowering=False, debug=False)

    char_ext = nc.dram_tensor("char", [BPC, N1, N2], I32, kind="ExternalInput").ap()
    # 4 strip-replicas of [block-diag W | bias row]: wb4[32g+c, cc*256+d] =
    # W[d]*(c==cc) for c<8, wb4[32g+8, cc*256+d] = b[d]
    wb4_ext = nc.dram_tensor("wb4", [105, NG * D], BF16, kind="ExternalInput").ap()
    const_ext = {
        name: nc.dram_tensor(name, list(arr.shape), F32, kind="ExternalInput").ap()
        for name, arr in make_consts().items()
    }
    # out[b, p, t, f] with s = 64*p + 16*t + f//256, d = f%256  — row-major
    # identical to (BPC, S, D)
    out_ext = nc.dram_tensor("out", [BPC, N1, 4, 4096], F32, kind="ExternalOutput").ap()

    with tile.TileContext(nc) as tc:
        with (
            tc.tile_pool(name="consts", bufs=1) as cpool,
            tc.tile_pool(name="work", bufs=2) as wpool,
            tc.tile_pool(name="stg", bufs=3) as spool,
            tc.tile_pool(name="ppa", bufs=1, space="PSUM") as ppa,
            tc.tile_pool(name="ppc", bufs=1, space="PSUM") as ppc,
            tc.tile_pool(name="ppy", bufs=2, space="PSUM") as ppy,
            tc.tile_pool(name="ppb", bufs=4, space="PSUM") as ppb,
        ):
            # ---- constants into SBUF ----
            cs = {}
            for name, ext in const_ext.items():
                t = cpool.tile(list(ext.shape), F32, name=f"c_{name}", tag=f"c_{name}")
                nc.sync.dma_start(out=t[:], in_=ext)
                cs[name] = t
            wb4 = cpool.tile([105, NG * D], BF16)
            nc.sync.dma_start(out=wb4[:], in_=wb4_ext)

            for bb in range(BPC):
                # ---- load + int->float ----
                char_sb = wpool.tile([N1, N2], I32, tag="char_sb")
                nc.sync.dma_start(out=char_sb[:], in_=char_ext[bb])
                xf = wpool.tile([N1, N2], F32, tag="xf")
                nc.vector.tensor_copy(xf[:], char_sb[:])

                # ---- MM1: A'[n2, f1] = Xm.T @ M1 (re | im packed in free) ----
                apack = ppa.tile([N2, 2 * N1], F32, tag="apack")
                are, aim = apack[:, 0:N1], apack[:, N1 : 2 * N1]
                nc.tensor.matmul(are, xf[:], cs["m1re"][:], start=True, stop=True)
                nc.tensor.matmul(aim, xf[:], cs["m1im"][:], start=True, stop=True)

                # ---- twiddle 1: B' = A' * TWT (complex) ----
                t1 = wpool.tile([N2, N1], F32, tag="t1")
                nc.vector.tensor_tensor(t1[:], are, cs["twtre"][:], MULT)
                t2 = wpool.tile([N2, N1], F32, tag="t2")
                nc.vector.tensor_tensor(t2[:], aim, cs["twtim"][:], MULT)
                bre = wpool.tile([N2, N1], F32, tag="bre")
                nc.vector.tensor_tensor(bre[:], t1[:], t2[:], SUB)
                t3 = wpool.tile([N2, N1], F32, tag="t3")
                nc.vector.tensor_tensor(t3[:], are, cs["twtim"][:], MULT)
                t4 = wpool.tile([N2, N1], F32, tag="t4")
                nc.vector.tensor_tensor(t4[:], aim, cs["twtre"][:], MULT)
                bim = wpool.tile([N2, N1], F32, tag="bim")
                nc.vector.tensor_tensor(bim[:], t3[:], t4[:], ADD)

                # ---- MM2: Ck[f1, m2'] = B'.T @ G (re | im packed in free) ----
                ckpack = ppc.tile([N1, 2 * N2], F32, tag="ckpack")
                ckre, ckim = ckpack[:, 0:N2], ckpack[:, N2 : 2 * N2]
                nc.tensor.matmul(ckre, bre[:], cs["gre"][:], start=True, stop=False)
                nc.tensor.matmul(ckre, bim[:], cs["gimn"][:], start=False, stop=True)
                nc.tensor.matmul(ckim, bre[:], cs["gim"][:], start=True, stop=False)
                nc.tensor.matmul(ckim, bim[:], cs["gre"][:], start=False, stop=True)

                # ---- twiddle 2: Dm = Ck * TW2, written into two (128,128)
                # tiles whose free dim is 4 strips of 32: [8 data cols | ones
                # col | 23 zero cols].  The ones col is (S,0,...) so MM3 emits
                # an exact ones row on that output partition. ----
                u1 = wpool.tile([N1, N2], F32, tag="u1")
                nc.vector.tensor_tensor(u1[:], ckre, cs["tw2re"][:], MULT)
                u2 = wpool.tile([N1, N2], F32, tag="u2")
                nc.vector.tensor_tensor(u2[:], ckim, cs["tw2im"][:], MULT)
                u3 = wpool.tile([N1, N2], F32, tag="u3")
                nc.vector.tensor_tensor(u3[:], ckre, cs["tw2im"][:], MULT)
                u4 = wpool.tile([N1, N2], F32, tag="u4")
                nc.vector.tensor_tensor(u4[:], ckim, cs["tw2re"][:], MULT)

                ylhs_half = []
                for half in range(2):
                    dmre = wpool.tile([N1, 128], F32, tag=f"dmre{half}")
                    dmim = wpool.tile([N1, 128], F32, tag=f"dmim{half}")
                    re3 = dmre.rearrange("p (g n) -> p g n", n=32)
                    im3 = dmim.rearrange("p (g n) -> p g n", n=32)
                    nc.gpsimd.memset(re3[:, :, NG:32], 0.0)
                    nc.gpsimd.memset(im3[:, :, NG:32], 0.0)
                    nc.gpsimd.memset(re3[0:1, :, NG : NG + 1], float(S))
                    cols = slice(32 * half, 32 * half + 32)
                    u13 = u1[:, cols].rearrange("p (g c) -> p g c", c=NG)
                    u23 = u2[:, cols].rearrange("p (g c) -> p g c", c=NG)
                    nc.vector.tensor_tensor(re3[:, :, 0:NG], u13, u23, SUB)
                    u33 = u3[:, cols].rearrange("p (g c) -> p g c", c=NG)
                    u43 = u4[:, cols].rearrange("p (g c) -> p g c", c=NG)
                    nc.vector.tensor_tensor(im3[:, :, 0:NG], u33, u43, ADD)

                    # ---- MM3: ylhs[32g+c, p] = y[64p + 8(4*half+g) + c],
                    # ylhs[32g+8, :] = 1 ----
                    ylhs_ps = ppy.tile([N1, N1], F32, tag="ylhs_ps")
                    nc.tensor.matmul(
                        ylhs_ps[:], dmre[:], cs["m3re"][:], start=True, stop=False
                    )
                    nc.tensor.matmul(
                        ylhs_ps[:], dmim[:], cs["m3imn"][:], start=False, stop=True
                    )
                    ylhs = wpool.tile([N1, N1], BF16, tag=f"ylhs{half}")
                    nc.vector.tensor_copy(ylhs[:], ylhs_ps[:])
                    ylhs_half.append(ylhs)

                # ---- broadcast: K=9 bf16 row-tiled matmuls, bias folded ----
                for t in range(4):
                    stg = spool.tile([N1, 16 * D], F32, tag="stg")
                    for h in range(2):
                        g = 2 * t + h          # group 0..7 -> chunks 8g..8g+7
                        ylhs = ylhs_half[g // 4]
                        gp = 32 * (g % 4)      # partition strip
                        rows = slice(gp, gp + NG + 1)
                        for q in range(4):
                            bcps = ppb.tile([N1, 512], F32, tag="bcps")
                            nc.tensor.matmul(
                                bcps[:],
                                ylhs[rows, :],
                                wb4[rows, 512 * q : 512 * (q + 1)],
                                start=True,
                                stop=True,
                                tile_position=(gp, 0),
                            )
                            dst = stg[:, 2048 * h + 512 * q : 2048 * h + 512 * (q + 1)]
                            if (h * 4 + q) % 2 == 0:
                                nc.scalar.copy(dst, bcps[:])
                            else:
                                nc.vector.tensor_copy(dst, bcps[:])
                    nc.sync.dma_start(out=out_ext[bb, :, t, :], in_=stg[:])

    nc.compile()
    return nc


_NC = None


def _get_nc():
    global _NC
    if _NC is None:
        _NC = build_program()
    return _NC


def make_in_maps(char_ids, W, b):
    char = np.ascontiguousarray(np.asarray(char_ids), dtype=np.int32)
    char = char.reshape(NCORES, BPC, N1, N2)
    wvec = np.asarray(W, dtype=np.float32)[:, 0]
    bvec = np.asarray(b, dtype=np.float32)
    wb9 = np.zeros((NG + 1, NG * D), dtype=np.float32)
    for c in range(NG):
        wb9[c, c * D : (c + 1) * D] = wvec
    wb9[NG] = np.tile(bvec, NG)
    wb4 = np.zeros((105, NG * D), dtype=np.float32)
    for g in range(4):
        wb4[32 * g : 32 * g + NG + 1] = wb9
    wb4 = wb4.astype(ml_dtypes.bfloat16)
    consts = make_consts()
    in_maps = []
    for i in range(NCORES):
        m = {"char": char[i], "wb4": wb4}
        m.update(consts)
        in_maps.append(m)
    return in_maps


def kernel(char_ids, W, b):
    nc = _get_nc()
    in_maps = make_in_maps(char_ids, W, b)
    res = run_bass_kernel_spmd(nc, in_maps, core_ids=list(range(NCORES)))
    parts = [r["out"].reshape(BPC, S, D) for r in res.results]
    return np.concatenate(parts, axis=0).astype(np.float32)
